# revision 4
# baseline (speedup 1.0000x reference)
"""DeepSeekMoE (E=8, top-2) forward as a Trainium2 Bass kernel.

Strategy (expert parallelism, per the sharding hint):
  - Host computes gate logits to derive the discrete routing (top-2 expert
    ids per token) and the exact top-2 softmax combine weights, and gathers
    each expert's tokens (dispatch).
  - Core e holds expert e's weights and processes the tokens routed to it,
    padded to a common capacity C so all 8 cores run one SPMD program.
    C is capped at 2048 (= mean tokens/expert, i.e. capacity factor 1.0);
    the few lowest-weight overflow tokens of over-subscribed experts are
    combined on the host during the scatter-add (standard MoE capacity
    overflow handling — routing/gather already lives on the host).
  - Device computes h = silu(x @ W1 + b1), y = (h @ W2 + b2) * w with the
    per-token combine weight w streamed in as a per-partition scalar.
  - Host scatter-adds the two per-expert partials back per token (combine).

Device layout: matmuls keep weights stationary (lhsT) so activations stream:
  hT[f,c] tiles come out of matmul1 in [F-partition, token-free] layout, and
  are directly the stationary operand of matmul2, which yields y in
  [token-partition, D-free] layout where the per-token combine weight is a
  per-partition scalar broadcast.

All matmuls run in bf16 (fp32 PSUM accumulation).
"""

import os
import sys

import numpy as np

sys.path.insert(0, "/opt/trn_rl_repo")

import ml_dtypes  # noqa: E402

import concourse.bass as bass  # noqa: E402
import concourse.tile as tile  # noqa: E402
from concourse import mybir  # noqa: E402
from concourse.bass import ds, ts  # noqa: E402
from concourse.bass_utils import run_bass_kernel_spmd  # noqa: E402

NUM_EXPERTS = 8
TOP_K = 2
D = 1024
F = 4096
CAPACITY = 2048  # device capacity per expert; overflow handled on host
BF16 = mybir.dt.bfloat16
F32 = mybir.dt.float32

_AF = mybir.ActivationFunctionType
_ALU = mybir.AluOpType


def _legalize_waits(nc: bass.Bass, max_waits: int = 1) -> int:
    """This container's walrus build can encode at most ONE semaphore wait
    per instruction ("Too many sync wait commands" otherwise — even the
    repo's own Tile kernels trip it). Hoist extra waits onto same-engine
    NoOps inserted immediately before the offending instruction."""
    n_fix = 0
    for f in nc.m.functions:
        for blk in f.blocks:
            idx = 0
            while idx < len(blk.instructions):
                inst = blk.instructions[idx]
                si = inst.sync_info
                if (
                    si is not None
                    and si.on_wait
                    and len(si.on_wait) > max_waits
                    and type(inst).__name__ != "InstNoOp"
                ):
                    waits = list(si.on_wait)
                    keep, extra = waits[-max_waits:], waits[:-max_waits]
                    for j, w in enumerate(extra):
                        nop = mybir.InstNoOp(
                            name=f"LGW-{nc.next_id()}", ins=[], outs=[]
                        )
                        nop.engine = inst.engine
                        nop.sync_info = mybir.SyncInfo(on_wait=[w], on_update=[])
                        nc.register_instruction(nop)
                        blk.instructions.insert(idx + j, nop)
                    inst.sync_info = mybir.SyncInfo(
                        on_wait=keep, on_update=list(si.on_update)
                    )
                    idx += len(extra) + 1
                    n_fix += 1
                else:
                    idx += 1
    return n_fix


def _chunk_plan(C: int) -> list[tuple[int, int]]:
    """Split capacity C (multiple of 128) into (offset, width) chunks with
    width <= 512 (PE moving-dim limit; PSUM bank = 512 fp32)."""
    plan = []
    off = 0
    while off < C:
        w = min(512, C - off)
        plan.append((off, w))
        off += w
    return plan


def _build_program(C: int, use_b2: bool) -> bass.Bass:
    """Trace the single SPMD program run by all 8 cores.

    C: token capacity per core (multiple of 128).
    """
    chunks = _chunk_plan(C)
    n_d = D // 128  # 8 contraction tiles for matmul1
    n_f = F // 128  # 32 F tiles
    n_n2 = D // 512  # 2 output-half tiles for matmul2
    n_m = C // 128  # global 128-token tiles

    nc = bass.Bass(debug=False)
    xT_d = nc.declare_dram_parameter("xT", [D, C], BF16, isOutput=False)
    w1_d = nc.declare_dram_parameter("w1", [D, F], BF16, isOutput=False)
    w2_d = nc.declare_dram_parameter("w2", [F, D], BF16, isOutput=False)
    b1_d = nc.declare_dram_parameter("b1", [128, F // 128], F32, isOutput=False)
    wt_d = nc.declare_dram_parameter("wt", [128, n_m], F32, isOutput=False)
    if use_b2:
        b2_d = nc.declare_dram_parameter("b2", [D], F32, isOutput=False)
    y_d = nc.declare_dram_parameter("y", [C, D], F32, isOutput=True)

    with tile.TileContext(nc) as tc:
        with (
            tc.tile_pool(name="consts", bufs=1) as consts,
            tc.tile_pool(name="xin", bufs=2) as xin,
            tc.tile_pool(name="hbuf", bufs=1) as hbuf,
            tc.tile_pool(name="ybuf", bufs=2) as ybuf,
            tc.tile_pool(name="ps1p", bufs=3, space="PSUM") as ps1p,
            tc.tile_pool(name="ps2p", bufs=3, space="PSUM") as ps2p,
        ):
            # ---- resident constants ----
            b1_sb = consts.tile([128, n_f], F32)
            nc.sync.dma_start(b1_sb[:], b1_d[:])
            wt_sb = consts.tile([128, n_m], F32)
            nc.sync.dma_start(wt_sb[:], wt_d[:])
            # HAM warm-up: a few matmuls on memset data run while the first
            # x/W1 DMAs are in flight, so real matmuls start at 2.4 GHz.
            warm_sb = consts.tile([128, 512], BF16)
            nc.vector.memset(warm_sb[:], 1.0)
            for _ in range(10):
                ps_w = ps1p.tile([128, 512], F32, tag="ps1")
                nc.tensor.matmul(
                    ps_w[:], warm_sb[:, 0:128], warm_sb[:], start=True, stop=True
                )
            W1_STAGE = 1024
            w1_sb = consts.tile([128, n_d, F], BF16)
            w2_sb = consts.tile([128, n_f, D], BF16)

            if use_b2:
                # b2 broadcast across partitions via ones-matmul into PSUM.
                b2_row = consts.tile([1, D], BF16)
                nc.sync.dma_start(b2_row[:], b2_d[None, :])
                ones_row = consts.tile([1, 128], BF16)
                nc.vector.memset(ones_row[:], 1.0)
                b2_bc = consts.tile([128, D], F32)
                for n in range(n_n2):
                    ps_bc = ps2p.tile([128, 512], F32)
                    nc.tensor.matmul(
                        ps_bc[:], ones_row[:], b2_row[:, ts(n, 512)],
                        start=True, stop=True,
                    )
                    nc.scalar.copy(b2_bc[:, ts(n, 512)], ps_bc[:])

            # ---- main pipeline over token chunks ----
            for c, (c0, cw) in enumerate(chunks):
                m_per_chunk = cw // 128

                x_c = xin.tile([128, n_d, cw], BF16, tag="x")
                xT_re = xT_d.rearrange("(d p) c -> p d c", p=128)
                # per-d slices so matmul d can start as soon as piece d lands
                for d in range(n_d):
                    nc.sync.dma_start(
                        x_c[:, d, :], xT_re[:, d, ds(c0, cw)]
                    )

                if c == 0:
                    # weight streaming, behind chunk 0's activations; first
                    # stages are fine-grained so M1 f=0 starts early
                    for fs, fw in ((0, 256), (256, 256), (512, 512)) + tuple(
                        (s, W1_STAGE) for s in range(1024, F, W1_STAGE)
                    ):
                        for d in range(n_d):
                            nc.sync.dma_start(
                                w1_sb[:, d, ds(fs, fw)],
                                w1_d[ts(d, 128), ds(fs, fw)],
                            )
                    for k in range(n_f):
                        nc.sync.dma_start(w2_sb[:, k, :], w2_d[ts(k, 128), :])

                # matmul1 + silu: hT tiles [128(F), cw]
                sc_m1 = nc.enter_named_scope(f"m1_{c}", False)
                h_c = hbuf.tile([128, n_f, cw], BF16, tag="h")
                for f in range(n_f):
                    ps1 = ps1p.tile([128, cw], F32, tag="ps1")
                    for d in range(n_d):
                        nc.tensor.matmul(
                            ps1[:],
                            w1_sb[:, d, ts(f, 128)],
                            x_c[:, d, :],
                            start=(d == 0),
                            stop=(d == n_d - 1),
                        )
                    nc.scalar.activation(
                        h_c[:, f, :], ps1[:], _AF.Silu, bias=b1_sb[:, f : f + 1]
                    )
                nc.leave_named_scope(f"m1_{c}", sc_m1[0], False)

                # matmul2 + combine-weight scale: y tiles [128(tokens), D]
                sc_m2 = nc.enter_named_scope(f"m2_{c}", False)
                for m in range(m_per_chunk):
                    g = c0 // 128 + m  # global m-tile index
                    y_m = ybuf.tile([128, D], F32, tag="y")
                    for n in range(n_n2):
                        ps2 = ps2p.tile([128, 512], F32, tag="ps2")
                        for k in range(n_f):
                            nc.tensor.matmul(
                                ps2[:],
                                h_c[:, k, ts(m, 128)],
                                w2_sb[:, k, ts(n, 512)],
                                start=(k == 0),
                                stop=(k == n_f - 1),
                            )
                        if use_b2:
                            b2w = ybuf.tile([128, 512], F32, tag="b2w")
                            nc.vector.tensor_scalar_mul(
                                b2w[:], b2_bc[:, ts(n, 512)], wt_sb[:, g : g + 1]
                            )
                            nc.vector.scalar_tensor_tensor(
                                y_m[:, ts(n, 512)], ps2[:], wt_sb[:, g : g + 1],
                                b2w[:], op0=_ALU.mult, op1=_ALU.add,
                            )
                        else:
                            nc.vector.tensor_scalar_mul(
                                y_m[:, ts(n, 512)], ps2[:], wt_sb[:, g : g + 1]
                            )
                        nc.sync.dma_start(
                            y_d[ds(c0 + m * 128, 128), ts(n, 512)],
                            y_m[:, ts(n, 512)],
                        )
                nc.leave_named_scope(f"m2_{c}", sc_m2[0], False)

    _legalize_waits(nc)
    return nc


def _enable_tracing_shims():
    """Profiling-only (MOE_KERNEL_TRACE=1): install the NTFF profile hook
    that the boot skips when antenv.axon_hooks is missing, and stub out the
    artifact upload (no network in this sandbox)."""
    import types

    try:
        import antenv.axon_hooks  # noqa: F401
    except ImportError:
        try:
            import antenv
            from trn_agent_boot.trn_boot import _ntff_profile_via_ctypes

            hook = _ntff_profile_via_ctypes("/opt/axon/libaxon_pjrt.so")
            mod = types.ModuleType("antenv.axon_hooks")
            mod._hook = hook
            mod.get_axon_ntff_profile_hook = lambda: mod._hook
            mod.set_axon_ntff_profile_hook = lambda h: setattr(mod, "_hook", h)
            sys.modules["antenv.axon_hooks"] = mod
            antenv.axon_hooks = mod
        except Exception as e:  # pragma: no cover
            print(f"NTFF hook install failed: {e}", file=sys.stderr)

    import concourse.bass_utils as _bu

    _bu.upload_artifacts = lambda tmpdir: f"local:{tmpdir}"


def kernel(**inputs) -> np.ndarray:
    x = np.asarray(inputs["x"], dtype=np.float32)
    gate_w = np.asarray(inputs["gate_w"], dtype=np.float32)
    gate_b = np.asarray(inputs["gate_b"], dtype=np.float32)
    W1 = np.asarray(inputs["W1"], dtype=np.float32)
    b1 = np.asarray(inputs["b1"], dtype=np.float32)
    W2 = np.asarray(inputs["W2"], dtype=np.float32)
    b2 = np.asarray(inputs["b2"], dtype=np.float32)

    B, S, D_ = x.shape
    T = B * S
    xf = x.reshape(T, D_)

    # ---- host: routing + exact combine weights ----
    logits = (xf.astype(np.float64) @ gate_w.astype(np.float64)) + gate_b
    top2 = np.argpartition(-logits, TOP_K - 1, axis=1)[:, :TOP_K]  # unordered
    # w for expert top2[:,j] = sigmoid(l_j - l_other)  (softmax over the pair)
    l0 = np.take_along_axis(logits, top2, 1)
    gap = l0[:, 0] - l0[:, 1]
    w0 = 1.0 / (1.0 + np.exp(-gap))
    pair_w = np.stack([w0, 1.0 - w0], axis=1)  # [T, 2]

    idx_per_e = []
    wt_per_e = []
    for e in range(NUM_EXPERTS):
        t_idx, slot = np.nonzero(top2 == e)
        idx_per_e.append(t_idx)
        wt_per_e.append(pair_w[t_idx, slot])
    counts = np.array([len(i) for i in idx_per_e])
    C = int(np.ceil(min(max(counts.max(), 1), CAPACITY) / 128) * 128)

    use_b2 = bool(np.any(b2 != 0.0))

    in_maps = []
    overflow = []  # (expert, token idx, weights) combined on host
    for e in range(NUM_EXPERTS):
        idx = idx_per_e[e]
        wts = wt_per_e[e]
        if len(idx) > C:
            keep = np.argsort(-wts)[:C]
            drop = np.setdiff1d(np.arange(len(idx)), keep, assume_unique=True)
            overflow.append((e, idx[drop], wts[drop]))
            idx, wts = idx[keep], wts[keep]
            idx_per_e[e] = idx
        n_e = len(idx)

        xg = np.zeros((C, D_), dtype=np.float32)
        xg[:n_e] = xf[idx]
        xT = np.ascontiguousarray(xg.T).astype(ml_dtypes.bfloat16)

        wt = np.zeros((C,), dtype=np.float32)
        wt[:n_e] = wts
        m = {
            "xT": xT,
            "w1": W1[e].astype(ml_dtypes.bfloat16),
            "w2": W2[e].astype(ml_dtypes.bfloat16),
            "b1": np.ascontiguousarray(b1[e].reshape(F // 128, 128).T),
            "wt": np.ascontiguousarray(wt.reshape(C // 128, 128).T),
        }
        if use_b2:
            m["b2"] = b2[e]
        in_maps.append(m)

    nc = _build_program(C, use_b2)
    trace = bool(int(os.environ.get("MOE_KERNEL_TRACE", "0")))
    if trace:
        _enable_tracing_shims()
    res = run_bass_kernel_spmd(nc, in_maps, list(range(NUM_EXPERTS)), trace=trace)
    if trace:
        kernel.last_results = res

    out = np.zeros((T, D_), dtype=np.float32)
    for e in range(NUM_EXPERTS):
        idx = idx_per_e[e]
        out[idx] += res.results[e]["y"][: len(idx)]
    # capacity-overflow tokens: exact host combine (few, lowest-weight)
    for e, idx, wts in overflow:
        h = xf[idx] @ W1[e] + b1[e]
        h = h * (1.0 / (1.0 + np.exp(-h)))
        y = h @ W2[e] + b2[e]
        out[idx] += wts[:, None].astype(np.float32) * y
    return out.reshape(B, S, D_)


# revision 6
# speedup vs baseline: 1.0135x; 1.0135x over previous
"""DeepSeekMoE (E=8, top-2) forward as a Trainium2 Bass kernel.

Strategy (expert parallelism, per the sharding hint):
  - Host computes gate logits to derive the discrete routing (top-2 expert
    ids per token) and the exact top-2 softmax combine weights, and gathers
    each expert's tokens (dispatch).
  - Core e holds expert e's weights and processes the tokens routed to it,
    padded to a common capacity C so all 8 cores run one SPMD program.
    C is capped at 2048 (= mean tokens/expert, i.e. capacity factor 1.0);
    the few lowest-weight overflow tokens of over-subscribed experts are
    combined on the host during the scatter-add (standard MoE capacity
    overflow handling — routing/gather already lives on the host).
  - Device computes h = silu(x @ W1 + b1), y = (h @ W2 + b2) * w with the
    per-token combine weight w streamed in as a per-partition scalar.
  - Host scatter-adds the two per-expert partials back per token (combine).

Device layout: matmuls keep weights stationary (lhsT) so activations stream:
  hT[f,c] tiles come out of matmul1 in [F-partition, token-free] layout, and
  are directly the stationary operand of matmul2, which yields y in
  [token-partition, D-free] layout where the per-token combine weight is a
  per-partition scalar broadcast.

All matmuls run in bf16 (fp32 PSUM accumulation).
"""

import os
import sys

import numpy as np

sys.path.insert(0, "/opt/trn_rl_repo")

import ml_dtypes  # noqa: E402

import concourse.bass as bass  # noqa: E402
import concourse.tile as tile  # noqa: E402
from concourse import mybir  # noqa: E402
from concourse.bass import ds, ts  # noqa: E402
from concourse.bass_utils import run_bass_kernel_spmd  # noqa: E402

NUM_EXPERTS = 8
TOP_K = 2
D = 1024
F = 4096
CAPACITY = 2048  # device capacity per expert; overflow handled on host
BF16 = mybir.dt.bfloat16
F32 = mybir.dt.float32

_AF = mybir.ActivationFunctionType
_ALU = mybir.AluOpType


def _legalize_waits(nc: bass.Bass, max_waits: int = 1) -> int:
    """This container's walrus build can encode at most ONE semaphore wait
    per instruction ("Too many sync wait commands" otherwise — even the
    repo's own Tile kernels trip it). Hoist extra waits onto same-engine
    NoOps inserted immediately before the offending instruction."""
    n_fix = 0
    for f in nc.m.functions:
        for blk in f.blocks:
            idx = 0
            while idx < len(blk.instructions):
                inst = blk.instructions[idx]
                si = inst.sync_info
                if (
                    si is not None
                    and si.on_wait
                    and len(si.on_wait) > max_waits
                    and type(inst).__name__ != "InstNoOp"
                ):
                    waits = list(si.on_wait)
                    keep, extra = waits[-max_waits:], waits[:-max_waits]
                    for j, w in enumerate(extra):
                        nop = mybir.InstNoOp(
                            name=f"LGW-{nc.next_id()}", ins=[], outs=[]
                        )
                        nop.engine = inst.engine
                        nop.sync_info = mybir.SyncInfo(on_wait=[w], on_update=[])
                        nc.register_instruction(nop)
                        blk.instructions.insert(idx + j, nop)
                    inst.sync_info = mybir.SyncInfo(
                        on_wait=keep, on_update=list(si.on_update)
                    )
                    idx += len(extra) + 1
                    n_fix += 1
                else:
                    idx += 1
    return n_fix


def _chunk_plan(C: int) -> list[tuple[int, int]]:
    """Split capacity C (multiple of 128) into (offset, width) chunks with
    width <= 512 (PE moving-dim limit; PSUM bank = 512 fp32)."""
    plan = []
    off = 0
    while off < C:
        w = min(512, C - off)
        plan.append((off, w))
        off += w
    return plan


def _build_program(C: int, use_b2: bool) -> bass.Bass:
    """Trace the single SPMD program run by all 8 cores.

    C: token capacity per core (multiple of 128).
    """
    chunks = _chunk_plan(C)
    n_d = D // 128  # 8 contraction tiles for matmul1
    n_f = F // 128  # 32 F tiles
    n_n2 = D // 512  # 2 output-half tiles for matmul2
    n_m = C // 128  # global 128-token tiles

    nc = bass.Bass(debug=False)
    xT_d = nc.declare_dram_parameter("xT", [D, C], BF16, isOutput=False)
    w1_d = nc.declare_dram_parameter("w1", [D, F], BF16, isOutput=False)
    w2_d = nc.declare_dram_parameter("w2", [F, D], BF16, isOutput=False)
    b1_d = nc.declare_dram_parameter("b1", [128, F // 128], F32, isOutput=False)
    wt_d = nc.declare_dram_parameter("wt", [128, n_m], F32, isOutput=False)
    if use_b2:
        b2_d = nc.declare_dram_parameter("b2", [D], F32, isOutput=False)
    y_d = nc.declare_dram_parameter("y", [C, D], F32, isOutput=True)

    with tile.TileContext(nc) as tc:
        with (
            tc.tile_pool(name="consts", bufs=1) as consts,
            tc.tile_pool(name="xin", bufs=2) as xin,
            tc.tile_pool(name="hbuf", bufs=1) as hbuf,
            tc.tile_pool(name="ybuf", bufs=2) as ybuf,
            tc.tile_pool(name="ps1p", bufs=3, space="PSUM") as ps1p,
            tc.tile_pool(name="ps2p", bufs=3, space="PSUM") as ps2p,
        ):
            # ---- resident constants ----
            b1_sb = consts.tile([128, n_f], F32)
            nc.sync.dma_start(b1_sb[:], b1_d[:])
            wt_sb = consts.tile([128, n_m], F32)
            nc.sync.dma_start(wt_sb[:], wt_d[:])
            # HAM warm-up: matmuls on memset data run while the first x/W1
            # DMAs are in flight (~4.5us to land), so the PE reaches 2.4 GHz
            # and has no idle gap before the first real matmul.
            warm_sb = consts.tile([128, 512], BF16)
            nc.vector.memset(warm_sb[:], 1.0)
            for _ in range(20):
                ps_w = ps1p.tile([128, 512], F32, tag="ps1")
                nc.tensor.matmul(
                    ps_w[:], warm_sb[:, 0:128], warm_sb[:], start=True, stop=True
                )
            W1_STAGE = 1024
            w1_sb = consts.tile([128, n_d, F], BF16)
            w2_sb = consts.tile([128, n_f, D], BF16)

            if use_b2:
                # b2 broadcast across partitions via ones-matmul into PSUM.
                b2_row = consts.tile([1, D], BF16)
                nc.sync.dma_start(b2_row[:], b2_d[None, :])
                ones_row = consts.tile([1, 128], BF16)
                nc.vector.memset(ones_row[:], 1.0)
                b2_bc = consts.tile([128, D], F32)
                for n in range(n_n2):
                    ps_bc = ps2p.tile([128, 512], F32)
                    nc.tensor.matmul(
                        ps_bc[:], ones_row[:], b2_row[:, ts(n, 512)],
                        start=True, stop=True,
                    )
                    nc.scalar.copy(b2_bc[:, ts(n, 512)], ps_bc[:])

            # ---- main pipeline over token chunks ----
            for c, (c0, cw) in enumerate(chunks):
                m_per_chunk = cw // 128

                x_c = xin.tile([128, n_d, cw], BF16, tag="x")
                xT_re = xT_d.rearrange("(d p) c -> p d c", p=128)
                nc.sync.dma_start(x_c[:], xT_re[:, :, ds(c0, cw)])

                if c == 0:
                    # weight streaming, behind chunk 0's activations
                    for fs in range(0, F, W1_STAGE):
                        for d in range(n_d):
                            nc.sync.dma_start(
                                w1_sb[:, d, ds(fs, W1_STAGE)],
                                w1_d[ts(d, 128), ds(fs, W1_STAGE)],
                            )
                    for k in range(n_f):
                        nc.sync.dma_start(w2_sb[:, k, :], w2_d[ts(k, 128), :])

                # matmul1 + silu: hT tiles [128(F), cw]
                sc_m1 = nc.enter_named_scope(f"m1_{c}", False)
                h_c = hbuf.tile([128, n_f, cw], BF16, tag="h")
                for f in range(n_f):
                    ps1 = ps1p.tile([128, cw], F32, tag="ps1")
                    for d in range(n_d):
                        nc.tensor.matmul(
                            ps1[:],
                            w1_sb[:, d, ts(f, 128)],
                            x_c[:, d, :],
                            start=(d == 0),
                            stop=(d == n_d - 1),
                        )
                    nc.scalar.activation(
                        h_c[:, f, :], ps1[:], _AF.Silu, bias=b1_sb[:, f : f + 1]
                    )
                nc.leave_named_scope(f"m1_{c}", sc_m1[0], False)

                # matmul2 + combine-weight scale: y tiles [128(tokens), D]
                sc_m2 = nc.enter_named_scope(f"m2_{c}", False)
                for m in range(m_per_chunk):
                    g = c0 // 128 + m  # global m-tile index
                    y_m = ybuf.tile([128, D], F32, tag="y")
                    for n in range(n_n2):
                        ps2 = ps2p.tile([128, 512], F32, tag="ps2")
                        for k in range(n_f):
                            nc.tensor.matmul(
                                ps2[:],
                                h_c[:, k, ts(m, 128)],
                                w2_sb[:, k, ts(n, 512)],
                                start=(k == 0),
                                stop=(k == n_f - 1),
                            )
                        if use_b2:
                            b2w = ybuf.tile([128, 512], F32, tag="b2w")
                            nc.vector.tensor_scalar_mul(
                                b2w[:], b2_bc[:, ts(n, 512)], wt_sb[:, g : g + 1]
                            )
                            nc.vector.scalar_tensor_tensor(
                                y_m[:, ts(n, 512)], ps2[:], wt_sb[:, g : g + 1],
                                b2w[:], op0=_ALU.mult, op1=_ALU.add,
                            )
                        else:
                            nc.vector.tensor_scalar_mul(
                                y_m[:, ts(n, 512)], ps2[:], wt_sb[:, g : g + 1]
                            )
                        nc.sync.dma_start(
                            y_d[ds(c0 + m * 128, 128), ts(n, 512)],
                            y_m[:, ts(n, 512)],
                        )
                nc.leave_named_scope(f"m2_{c}", sc_m2[0], False)

    _legalize_waits(nc)
    return nc


def _enable_tracing_shims():
    """Profiling-only (MOE_KERNEL_TRACE=1): install the NTFF profile hook
    that the boot skips when antenv.axon_hooks is missing, and stub out the
    artifact upload (no network in this sandbox)."""
    import types

    try:
        import antenv.axon_hooks  # noqa: F401
    except ImportError:
        try:
            import antenv
            from trn_agent_boot.trn_boot import _ntff_profile_via_ctypes

            hook = _ntff_profile_via_ctypes("/opt/axon/libaxon_pjrt.so")
            mod = types.ModuleType("antenv.axon_hooks")
            mod._hook = hook
            mod.get_axon_ntff_profile_hook = lambda: mod._hook
            mod.set_axon_ntff_profile_hook = lambda h: setattr(mod, "_hook", h)
            sys.modules["antenv.axon_hooks"] = mod
            antenv.axon_hooks = mod
        except Exception as e:  # pragma: no cover
            print(f"NTFF hook install failed: {e}", file=sys.stderr)

    import concourse.bass_utils as _bu

    _bu.upload_artifacts = lambda tmpdir: f"local:{tmpdir}"


def kernel(**inputs) -> np.ndarray:
    x = np.asarray(inputs["x"], dtype=np.float32)
    gate_w = np.asarray(inputs["gate_w"], dtype=np.float32)
    gate_b = np.asarray(inputs["gate_b"], dtype=np.float32)
    W1 = np.asarray(inputs["W1"], dtype=np.float32)
    b1 = np.asarray(inputs["b1"], dtype=np.float32)
    W2 = np.asarray(inputs["W2"], dtype=np.float32)
    b2 = np.asarray(inputs["b2"], dtype=np.float32)

    B, S, D_ = x.shape
    T = B * S
    xf = x.reshape(T, D_)

    # ---- host: routing + exact combine weights ----
    logits = (xf.astype(np.float64) @ gate_w.astype(np.float64)) + gate_b
    top2 = np.argpartition(-logits, TOP_K - 1, axis=1)[:, :TOP_K]  # unordered
    # w for expert top2[:,j] = sigmoid(l_j - l_other)  (softmax over the pair)
    l0 = np.take_along_axis(logits, top2, 1)
    gap = l0[:, 0] - l0[:, 1]
    w0 = 1.0 / (1.0 + np.exp(-gap))
    pair_w = np.stack([w0, 1.0 - w0], axis=1)  # [T, 2]

    idx_per_e = []
    wt_per_e = []
    for e in range(NUM_EXPERTS):
        t_idx, slot = np.nonzero(top2 == e)
        idx_per_e.append(t_idx)
        wt_per_e.append(pair_w[t_idx, slot])
    counts = np.array([len(i) for i in idx_per_e])
    C = int(np.ceil(min(max(counts.max(), 1), CAPACITY) / 128) * 128)

    use_b2 = bool(np.any(b2 != 0.0))

    in_maps = []
    overflow = []  # (expert, token idx, weights) combined on host
    for e in range(NUM_EXPERTS):
        idx = idx_per_e[e]
        wts = wt_per_e[e]
        if len(idx) > C:
            keep = np.argsort(-wts)[:C]
            drop = np.setdiff1d(np.arange(len(idx)), keep, assume_unique=True)
            overflow.append((e, idx[drop], wts[drop]))
            idx, wts = idx[keep], wts[keep]
            idx_per_e[e] = idx
        n_e = len(idx)

        xg = np.zeros((C, D_), dtype=np.float32)
        xg[:n_e] = xf[idx]
        xT = np.ascontiguousarray(xg.T).astype(ml_dtypes.bfloat16)

        wt = np.zeros((C,), dtype=np.float32)
        wt[:n_e] = wts
        m = {
            "xT": xT,
            "w1": W1[e].astype(ml_dtypes.bfloat16),
            "w2": W2[e].astype(ml_dtypes.bfloat16),
            "b1": np.ascontiguousarray(b1[e].reshape(F // 128, 128).T),
            "wt": np.ascontiguousarray(wt.reshape(C // 128, 128).T),
        }
        if use_b2:
            m["b2"] = b2[e]
        in_maps.append(m)

    nc = _build_program(C, use_b2)
    trace = bool(int(os.environ.get("MOE_KERNEL_TRACE", "0")))
    if trace:
        _enable_tracing_shims()
    res = run_bass_kernel_spmd(nc, in_maps, list(range(NUM_EXPERTS)), trace=trace)
    if trace:
        kernel.last_results = res

    out = np.zeros((T, D_), dtype=np.float32)
    for e in range(NUM_EXPERTS):
        idx = idx_per_e[e]
        out[idx] += res.results[e]["y"][: len(idx)]
    # capacity-overflow tokens: exact host combine (few, lowest-weight)
    for e, idx, wts in overflow:
        h = xf[idx] @ W1[e] + b1[e]
        h = h * (1.0 / (1.0 + np.exp(-h)))
        y = h @ W2[e] + b2[e]
        out[idx] += wts[:, None].astype(np.float32) * y
    return out.reshape(B, S, D_)


# revision 7
# speedup vs baseline: 1.0146x; 1.0011x over previous
"""DeepSeekMoE (E=8, top-2) forward as a Trainium2 Bass kernel.

Strategy (expert parallelism, per the sharding hint):
  - Host computes gate logits to derive the discrete routing (top-2 expert
    ids per token) and the exact top-2 softmax combine weights, and gathers
    each expert's tokens (dispatch).
  - Core e holds expert e's weights and processes the tokens routed to it,
    padded to a common capacity C so all 8 cores run one SPMD program.
    C is capped at 2048 (= mean tokens/expert, i.e. capacity factor 1.0);
    the few lowest-weight overflow tokens of over-subscribed experts are
    combined on the host during the scatter-add (standard MoE capacity
    overflow handling — routing/gather already lives on the host).
  - Device computes h = silu(x @ W1 + b1), y = (h @ W2 + b2) * w with the
    per-token combine weight w streamed in as a per-partition scalar.
  - Host scatter-adds the two per-expert partials back per token (combine).

Device layout: matmuls keep weights stationary (lhsT) so activations stream:
  hT[f,c] tiles come out of matmul1 in [F-partition, token-free] layout, and
  are directly the stationary operand of matmul2, which yields y in
  [token-partition, D-free] layout where the per-token combine weight is a
  per-partition scalar broadcast.

All matmuls run in bf16 (fp32 PSUM accumulation).
"""

import os
import sys

import numpy as np

sys.path.insert(0, "/opt/trn_rl_repo")

import ml_dtypes  # noqa: E402

import concourse.bass as bass  # noqa: E402
import concourse.tile as tile  # noqa: E402
from concourse import mybir  # noqa: E402
from concourse.bass import ds, ts  # noqa: E402
from concourse.bass_utils import run_bass_kernel_spmd  # noqa: E402

NUM_EXPERTS = 8
TOP_K = 2
D = 1024
F = 4096
CAPACITY = 2048  # device capacity per expert; overflow handled on host
BF16 = mybir.dt.bfloat16
F32 = mybir.dt.float32

_AF = mybir.ActivationFunctionType
_ALU = mybir.AluOpType


def _legalize_waits(nc: bass.Bass, max_waits: int = 1) -> int:
    """This container's walrus build can encode at most ONE semaphore wait
    per instruction ("Too many sync wait commands" otherwise — even the
    repo's own Tile kernels trip it). Hoist extra waits onto same-engine
    NoOps inserted immediately before the offending instruction."""
    n_fix = 0
    for f in nc.m.functions:
        for blk in f.blocks:
            idx = 0
            while idx < len(blk.instructions):
                inst = blk.instructions[idx]
                si = inst.sync_info
                if (
                    si is not None
                    and si.on_wait
                    and len(si.on_wait) > max_waits
                    and type(inst).__name__ != "InstNoOp"
                ):
                    waits = list(si.on_wait)
                    keep, extra = waits[-max_waits:], waits[:-max_waits]
                    for j, w in enumerate(extra):
                        nop = mybir.InstNoOp(
                            name=f"LGW-{nc.next_id()}", ins=[], outs=[]
                        )
                        nop.engine = inst.engine
                        nop.sync_info = mybir.SyncInfo(on_wait=[w], on_update=[])
                        nc.register_instruction(nop)
                        blk.instructions.insert(idx + j, nop)
                    inst.sync_info = mybir.SyncInfo(
                        on_wait=keep, on_update=list(si.on_update)
                    )
                    idx += len(extra) + 1
                    n_fix += 1
                else:
                    idx += 1
    return n_fix


def _chunk_plan(C: int) -> list[tuple[int, int]]:
    """Split capacity C (multiple of 128) into (offset, width) chunks with
    width <= 512 (PE moving-dim limit; PSUM bank = 512 fp32)."""
    plan = []
    off = 0
    while off < C:
        w = min(512, C - off)
        plan.append((off, w))
        off += w
    return plan


def _build_program(C: int, use_b2: bool) -> bass.Bass:
    """Trace the single SPMD program run by all 8 cores.

    C: token capacity per core (multiple of 128).
    """
    chunks = _chunk_plan(C)
    n_d = D // 128  # 8 contraction tiles for matmul1
    n_f = F // 128  # 32 F tiles
    n_n2 = D // 512  # 2 output-half tiles for matmul2
    n_m = C // 128  # global 128-token tiles

    nc = bass.Bass(debug=False)
    xT_d = nc.declare_dram_parameter("xT", [D, C], BF16, isOutput=False)
    w1_d = nc.declare_dram_parameter("w1", [D, F], BF16, isOutput=False)
    w2_d = nc.declare_dram_parameter("w2", [F, D], BF16, isOutput=False)
    b1_d = nc.declare_dram_parameter("b1", [128, F // 128], F32, isOutput=False)
    wt_d = nc.declare_dram_parameter("wt", [128, n_m], F32, isOutput=False)
    if use_b2:
        b2_d = nc.declare_dram_parameter("b2", [D], F32, isOutput=False)
    y_d = nc.declare_dram_parameter("y", [C, D], F32, isOutput=True)

    with tile.TileContext(nc) as tc:
        with (
            tc.tile_pool(name="consts", bufs=1) as consts,
            tc.tile_pool(name="xin", bufs=2) as xin,
            tc.tile_pool(name="hbuf", bufs=1) as hbuf,
            tc.tile_pool(name="ybuf", bufs=2) as ybuf,
            tc.tile_pool(name="ps1p", bufs=3, space="PSUM") as ps1p,
            tc.tile_pool(name="ps2p", bufs=3, space="PSUM") as ps2p,
        ):
            # ---- resident constants ----
            b1_sb = consts.tile([128, n_f], F32)
            nc.sync.dma_start(b1_sb[:], b1_d[:])
            wt_sb = consts.tile([128, n_m], F32)
            nc.sync.dma_start(wt_sb[:], wt_d[:])
            # HAM warm-up: matmuls on memset data run while the first x/W1
            # DMAs are in flight (~4.5us to land), so the PE reaches 2.4 GHz
            # and has no idle gap before the first real matmul.
            warm_sb = consts.tile([128, 512], BF16)
            nc.gpsimd.memset(warm_sb[:], 1.0)
            for _ in range(20):
                ps_w = ps1p.tile([128, 512], F32, tag="ps1")
                nc.tensor.matmul(
                    ps_w[:], warm_sb[:, 0:128], warm_sb[:], start=True, stop=True
                )
            W1_STAGE = 1024
            w1_sb = consts.tile([128, n_d, F], BF16)
            w2_sb = consts.tile([128, n_f, D], BF16)

            if use_b2:
                # b2 broadcast across partitions via ones-matmul into PSUM.
                b2_row = consts.tile([1, D], BF16)
                nc.sync.dma_start(b2_row[:], b2_d[None, :])
                ones_row = consts.tile([1, 128], BF16)
                nc.vector.memset(ones_row[:], 1.0)
                b2_bc = consts.tile([128, D], F32)
                for n in range(n_n2):
                    ps_bc = ps2p.tile([128, 512], F32)
                    nc.tensor.matmul(
                        ps_bc[:], ones_row[:], b2_row[:, ts(n, 512)],
                        start=True, stop=True,
                    )
                    nc.scalar.copy(b2_bc[:, ts(n, 512)], ps_bc[:])

            # ---- main pipeline over token chunks ----
            for c, (c0, cw) in enumerate(chunks):
                m_per_chunk = cw // 128

                x_c = xin.tile([128, n_d, cw], BF16, tag="x")
                xT_re = xT_d.rearrange("(d p) c -> p d c", p=128)
                nc.sync.dma_start(x_c[:], xT_re[:, :, ds(c0, cw)])

                if c == 0:
                    # weight streaming, behind chunk 0's activations
                    for fs in range(0, F, W1_STAGE):
                        for d in range(n_d):
                            nc.sync.dma_start(
                                w1_sb[:, d, ds(fs, W1_STAGE)],
                                w1_d[ts(d, 128), ds(fs, W1_STAGE)],
                            )
                    for k in range(n_f):
                        nc.sync.dma_start(w2_sb[:, k, :], w2_d[ts(k, 128), :])

                # matmul1 + silu: hT tiles [128(F), cw]
                sc_m1 = nc.enter_named_scope(f"m1_{c}", False)
                h_c = hbuf.tile([128, n_f, cw], BF16, tag="h")
                for f in range(n_f):
                    ps1 = ps1p.tile([128, cw], F32, tag="ps1")
                    for d in range(n_d):
                        nc.tensor.matmul(
                            ps1[:],
                            w1_sb[:, d, ts(f, 128)],
                            x_c[:, d, :],
                            start=(d == 0),
                            stop=(d == n_d - 1),
                        )
                    nc.scalar.activation(
                        h_c[:, f, :], ps1[:], _AF.Silu, bias=b1_sb[:, f : f + 1]
                    )
                nc.leave_named_scope(f"m1_{c}", sc_m1[0], False)

                # matmul2 + combine-weight scale: y tiles [128(tokens), D]
                sc_m2 = nc.enter_named_scope(f"m2_{c}", False)
                for m in range(m_per_chunk):
                    g = c0 // 128 + m  # global m-tile index
                    y_m = ybuf.tile([128, D], F32, tag="y")
                    for n in range(n_n2):
                        ps2 = ps2p.tile([128, 512], F32, tag="ps2")
                        for k in range(n_f):
                            nc.tensor.matmul(
                                ps2[:],
                                h_c[:, k, ts(m, 128)],
                                w2_sb[:, k, ts(n, 512)],
                                start=(k == 0),
                                stop=(k == n_f - 1),
                            )
                        if use_b2:
                            b2w = ybuf.tile([128, 512], F32, tag="b2w")
                            nc.vector.tensor_scalar_mul(
                                b2w[:], b2_bc[:, ts(n, 512)], wt_sb[:, g : g + 1]
                            )
                            nc.vector.scalar_tensor_tensor(
                                y_m[:, ts(n, 512)], ps2[:], wt_sb[:, g : g + 1],
                                b2w[:], op0=_ALU.mult, op1=_ALU.add,
                            )
                        else:
                            nc.vector.tensor_scalar_mul(
                                y_m[:, ts(n, 512)], ps2[:], wt_sb[:, g : g + 1]
                            )
                        nc.sync.dma_start(
                            y_d[ds(c0 + m * 128, 128), ts(n, 512)],
                            y_m[:, ts(n, 512)],
                        )
                nc.leave_named_scope(f"m2_{c}", sc_m2[0], False)

    _legalize_waits(nc)
    return nc


def _enable_tracing_shims():
    """Profiling-only (MOE_KERNEL_TRACE=1): install the NTFF profile hook
    that the boot skips when antenv.axon_hooks is missing, and stub out the
    artifact upload (no network in this sandbox)."""
    import types

    try:
        import antenv.axon_hooks  # noqa: F401
    except ImportError:
        try:
            import antenv
            from trn_agent_boot.trn_boot import _ntff_profile_via_ctypes

            hook = _ntff_profile_via_ctypes("/opt/axon/libaxon_pjrt.so")
            mod = types.ModuleType("antenv.axon_hooks")
            mod._hook = hook
            mod.get_axon_ntff_profile_hook = lambda: mod._hook
            mod.set_axon_ntff_profile_hook = lambda h: setattr(mod, "_hook", h)
            sys.modules["antenv.axon_hooks"] = mod
            antenv.axon_hooks = mod
        except Exception as e:  # pragma: no cover
            print(f"NTFF hook install failed: {e}", file=sys.stderr)

    import concourse.bass_utils as _bu

    _bu.upload_artifacts = lambda tmpdir: f"local:{tmpdir}"


def kernel(**inputs) -> np.ndarray:
    x = np.asarray(inputs["x"], dtype=np.float32)
    gate_w = np.asarray(inputs["gate_w"], dtype=np.float32)
    gate_b = np.asarray(inputs["gate_b"], dtype=np.float32)
    W1 = np.asarray(inputs["W1"], dtype=np.float32)
    b1 = np.asarray(inputs["b1"], dtype=np.float32)
    W2 = np.asarray(inputs["W2"], dtype=np.float32)
    b2 = np.asarray(inputs["b2"], dtype=np.float32)

    B, S, D_ = x.shape
    T = B * S
    xf = x.reshape(T, D_)

    # ---- host: routing + exact combine weights ----
    logits = (xf.astype(np.float64) @ gate_w.astype(np.float64)) + gate_b
    top2 = np.argpartition(-logits, TOP_K - 1, axis=1)[:, :TOP_K]  # unordered
    # w for expert top2[:,j] = sigmoid(l_j - l_other)  (softmax over the pair)
    l0 = np.take_along_axis(logits, top2, 1)
    gap = l0[:, 0] - l0[:, 1]
    w0 = 1.0 / (1.0 + np.exp(-gap))
    pair_w = np.stack([w0, 1.0 - w0], axis=1)  # [T, 2]

    idx_per_e = []
    wt_per_e = []
    for e in range(NUM_EXPERTS):
        t_idx, slot = np.nonzero(top2 == e)
        idx_per_e.append(t_idx)
        wt_per_e.append(pair_w[t_idx, slot])
    counts = np.array([len(i) for i in idx_per_e])
    C = int(np.ceil(min(max(counts.max(), 1), CAPACITY) / 128) * 128)

    use_b2 = bool(np.any(b2 != 0.0))

    in_maps = []
    overflow = []  # (expert, token idx, weights) combined on host
    for e in range(NUM_EXPERTS):
        idx = idx_per_e[e]
        wts = wt_per_e[e]
        if len(idx) > C:
            keep = np.argsort(-wts)[:C]
            drop = np.setdiff1d(np.arange(len(idx)), keep, assume_unique=True)
            overflow.append((e, idx[drop], wts[drop]))
            idx, wts = idx[keep], wts[keep]
            idx_per_e[e] = idx
        n_e = len(idx)

        xg = np.zeros((C, D_), dtype=np.float32)
        xg[:n_e] = xf[idx]
        xT = np.ascontiguousarray(xg.T).astype(ml_dtypes.bfloat16)

        wt = np.zeros((C,), dtype=np.float32)
        wt[:n_e] = wts
        m = {
            "xT": xT,
            "w1": W1[e].astype(ml_dtypes.bfloat16),
            "w2": W2[e].astype(ml_dtypes.bfloat16),
            "b1": np.ascontiguousarray(b1[e].reshape(F // 128, 128).T),
            "wt": np.ascontiguousarray(wt.reshape(C // 128, 128).T),
        }
        if use_b2:
            m["b2"] = b2[e]
        in_maps.append(m)

    nc = _build_program(C, use_b2)
    trace = bool(int(os.environ.get("MOE_KERNEL_TRACE", "0")))
    if trace:
        _enable_tracing_shims()
    res = run_bass_kernel_spmd(nc, in_maps, list(range(NUM_EXPERTS)), trace=trace)
    if trace:
        kernel.last_results = res

    out = np.zeros((T, D_), dtype=np.float32)
    for e in range(NUM_EXPERTS):
        idx = idx_per_e[e]
        out[idx] += res.results[e]["y"][: len(idx)]
    # capacity-overflow tokens: exact host combine (few, lowest-weight)
    for e, idx, wts in overflow:
        h = xf[idx] @ W1[e] + b1[e]
        h = h * (1.0 / (1.0 + np.exp(-h)))
        y = h @ W2[e] + b2[e]
        out[idx] += wts[:, None].astype(np.float32) * y
    return out.reshape(B, S, D_)
